# revision 51
# baseline (speedup 1.0000x reference)
"""GAT (2-layer graph attention network) Trainium2 Bass kernel, 8-core SPMD.

Sharding: core c computes head c of layer 1 (head-parallel) and rows
[c*256,(c+1)*256) of each half of the single-head output layer
(row-parallel), with a ReduceScatter+AllGather exchange of the per-head
h @ Wo partial products per half.

Key identity: with s = f_src[i] + f_dst[j],
  exp(leaky_relu(s)) = exp(f_src[i]) * exp(f_dst[j]) * max(exp((a-1)s), 1)
for 0<a<=1. The exp(f_src[i]) factor cancels in the softmax normalization,
exp(f_dst[j]) folds into per-partition scalars, so the N^2 inner loop is ONE
DVE tensor_scalar (mult+max, 4x perf mode) building
  u[j,i] = max(R[i]*(r*e1)[j], e1[j]),  R = exp((a-1) f_src),
  r = exp((a-1) f_dst), e1 = exp(f_dst - C),
one DVE tensor_tensor mask-multiply u*adj (2x), and the PE matmul against
unscaled weights [Wh | 1] (the ones column produces the softmax denominator).
All N^2 tensors are bf16: enough range that no clamps/shifts are needed.

Pipelining: layer 1 runs in two column pieces; each piece's normalization /
ELU / h@Wo / collective tail overlaps the next piece (piece-A tail runs on
the idle Pool engine so DVE keeps streaming attention). Layer 2 is split by
exchange piece in both j (weights) and i (columns) so work starts as soon as
each ReduceScatter / AllGather lands. Adjacency loads stream on the SP DMA
queue; small stores ride the ACT queue so they never head-block the stream.

kernel(**inputs) takes full unsharded inputs, returns the full output.
"""

from contextlib import ExitStack

import numpy as np
import ml_dtypes

import concourse.mybir as mybir
import concourse.tile as tile
from concourse import bacc
from concourse.bass_utils import run_bass_kernel_spmd

# Steer every activation to the one ACT table set covering all functions this
# kernel uses (Exp, Identity/Copy, Ln) so no mid-kernel table reloads happen.
_orig_get_tables = bacc.get_activation_tables


def _pinned_tables(arch):
    tabs = _orig_get_tables(arch)
    if "natural_log_exp_and_others" in tabs:
        return {name: (funcs if name == "natural_log_exp_and_others" else set())
                for name, funcs in tabs.items()}
    return tabs


bacc.get_activation_tables = _pinned_tables

N = 4096
F = 512
D = 64          # per-head hidden == n classes
H = 8
ALPHA = 0.2
N_CORES = 8
P = 128
NT = N // P             # 32 node tiles
SLICE = N // N_CORES    # 512 rows per core in layer 2
NKF = F // P            # 4 contraction tiles for x @ W
C_DST = 2.0             # conditioning shift on e1 (cancels in softmax)
PIECES = [2048, 2048]         # layer-1 attention column pieces (exchange units)
NP = len(PIECES)
P_OFF = [sum(PIECES[:h]) for h in range(NP)]
P_TILES = [c // P for c in PIECES]
RS_ROWS = [c // N_CORES for c in PIECES]    # per-core layer-2 rows per piece
RS_OFF = [sum(RS_ROWS[:h]) for h in range(NP)]
L2_JT = P_TILES               # whs2 tiles per exchange piece
WBIG = max(PIECES)

F32 = mybir.dt.float32
F16 = mybir.dt.float16
BF16 = mybir.dt.bfloat16

_CACHED = {}
POOL_FRAC = 0


def _att_rows(nc, pools, tag, whs_big, e1_sb, re1_sb, R_bc, r_lo, j_range,
              adj_fn, accs, cw, j_start, j_stop, pool_frac=0):
    """One attention sub-loop: for j in j_range, build the masked unnormalized
    attention tile v[j*, i] and accumulate whs^T @ v into accs (chunks of cw).
    R_bc columns [r_lo, r_lo+len(accs)*cw) select the i-slice. adj_fn(j) must
    return a bf16 AP [P, width] holding adj^T rows j*P..(j+1)*P for the same
    i-slice (may be SBUF-resident or a fresh DMA tile; it is not clobbered).
    Every pool_frac-th mask-multiply runs on the otherwise-idle Pool engine
    (slower per element, but off the critical DVE stream)."""
    work = pools["work"]
    width = len(accs) * cw

    def emit_u(j):
        u_t = work.tile([P, WBIG if width > 512 else 512], BF16,
                        tag="ubig" if width > 512 else "u512",
                        bufs=(4 if width > 512 else 4),
                        name=f"u_{tag}_{j}")
        u_t = u_t[:, 0:width]
        nc.vector.tensor_scalar(u_t[:], R_bc[:, r_lo:r_lo + width],
                                re1_sb[:, j:j + 1], e1_sb[:, j:j + 1],
                                mybir.AluOpType.mult, mybir.AluOpType.max)
        return u_t

    def emit_v(j, u_t, on_pool):
        adj_ap = adj_fn(j)
        v_t = work.tile([P, WBIG if width > 512 else 512], BF16,
                        tag="vbig" if width > 512 else "v512",
                        bufs=(6 if width > 512 else 6),
                        name=f"v_{tag}_{j}")
        v_t = v_t[:, 0:width]
        (nc.gpsimd.tensor_mul if on_pool else nc.vector.tensor_mul)(
            v_t[:], u_t[:], adj_ap)
        return v_t

    def emit_mm(j, v_t):
        for q, acc in enumerate(accs):
            nc.tensor.matmul(acc[:],
                             whs_big[:, j * (D + 1):(j + 1) * (D + 1)],
                             v_t[:, q * cw:(q + 1) * cw],
                             start=(j == j_start), stop=(j == j_stop))

    js = list(j_range)
    g = pool_frac if pool_frac else 1
    for g0 in range(0, len(js), g):
        grp = js[g0:g0 + g]
        vs = {}
        if pool_frac and len(grp) == g:
            # the Pool-handled j goes first so its slow multiply overlaps
            # the DVE work on the rest of the group
            jp = grp[-1]
            up = emit_u(jp)
            vs[jp] = emit_v(jp, up, True)
        for j in grp[:-1] if (pool_frac and len(grp) == g) else grp:
            u_t = emit_u(j)
            vs[j] = emit_v(j, u_t, False)
        for j in grp:
            emit_mm(j, vs[j])


def _norm(nc, pools, tag, accs, out_sb, o_lo, cw, on_pool):
    """Softmax-normalize accumulated chunks: out = num / den, den from the
    ones-column row D. For a hidden (non-final) piece the copies ride ACT and
    the multiplies Pool so the DVE attention stream is untouched; the final
    piece uses DVE for minimum latency into the collective."""
    psum, work = pools["psum"], pools["work"]
    ones64t = _CACHED["ones64t"]
    for q, acc in enumerate(accs):
        num_sb = work.tile([D, 512], BF16, tag="num", bufs=4,
                           name=f"num_{tag}_{q}")[:, 0:cw]
        den_sb = work.tile([1, 512], F32, tag="den", bufs=4,
                           name=f"den_{tag}_{q}")[:, 0:cw]
        nc.vector.tensor_copy(den_sb[:], acc[D:D + 1, :])
        nc.vector.tensor_copy(num_sb[:], acc[0:D, :])
        den_bc = psum.tile([D, cw], F32, tag="bank", bufs=8,
                           name=f"denbc_{tag}_{q}")
        nc.tensor.matmul(den_bc[:], ones64t[0:1, :], den_sb[:],
                         start=True, stop=True)
        recb = work.tile([D, 512], F32, tag="recb", bufs=1,
                         name=f"recb_{tag}_{q}")[:, 0:cw]
        nc.vector.reciprocal_approx_fast(recb[:], den_bc[:])
        mul = nc.gpsimd.tensor_mul if on_pool else nc.vector.tensor_mul
        mul(out_sb[:, o_lo + q * cw:o_lo + (q + 1) * cw],
            num_sb[:], recb[:])


def _elu(nc, pools, tag, src_ap, dst_ap, width, on_pool, ew=512):
    """dst = elu(src) elementwise on [D, width] tiles:
    elu(x) = exp(min(x,0)) + (max(x,0) - 1)."""
    work = pools["work"]
    ve = nc.gpsimd if on_pool else nc.vector
    for s in range(width // ew):
        sl = slice(s * ew, (s + 1) * ew)
        t_min = work.tile([D, ew], BF16, tag="elu_min", bufs=2,
                          name=f"elmin_{tag}_{s}")
        ve.tensor_scalar(t_min[:], src_ap[:, sl], 0.0, None,
                         mybir.AluOpType.min)
        t_exp = work.tile([D, ew], BF16, tag="elu_exp", bufs=2,
                          name=f"elexp_{tag}_{s}")
        nc.scalar.activation(t_exp[:], t_min[:],
                             mybir.ActivationFunctionType.Exp)
        t_lin = work.tile([D, ew], BF16, tag="elu_lin", bufs=2,
                          name=f"ellin_{tag}_{s}")
        ve.tensor_scalar(t_lin[:], src_ap[:, sl], 0.0, -1.0,
                         mybir.AluOpType.max, mybir.AluOpType.add)
        ve.tensor_add(dst_ap[:, sl], t_exp[:], t_lin[:])


def _tile_prep(nc, pools, tag, t, src_ap, whs_big, e1_sb, re1_sb):
    """Per node tile: whs block [Wh | 1] plus the two exp scalars from the
    staged f_dst column (src_ap [P, D+2] = [Wh | f_src | f_dst])."""
    nc.scalar.activation(e1_sb[:, t:t + 1], src_ap[:, D + 1:D + 2],
                         mybir.ActivationFunctionType.Exp,
                         bias=_CACHED["bias_d"][:])
    nc.scalar.activation(re1_sb[:, t:t + 1], src_ap[:, D + 1:D + 2],
                         mybir.ActivationFunctionType.Exp,
                         scale=ALPHA, bias=_CACHED["bias_d"][:])
    nc.scalar.activation(whs_big[:, t * (D + 1):t * (D + 1) + D],
                         src_ap[:, 0:D],
                         mybir.ActivationFunctionType.Copy)


def build_kernel(repeat=1):
    nc = bacc.Bacc("TRN2", num_devices=N_CORES)

    xT = nc.dram_tensor("xT", [F, N], F16, kind="ExternalInput")
    adjT = nc.dram_tensor("adjT", [N, N], BF16, kind="ExternalInput")
    adjT2 = nc.dram_tensor("adjT2", [N, SLICE], BF16, kind="ExternalInput")
    Wext = nc.dram_tensor("Wext", [F, D + 2], F16, kind="ExternalInput")
    Woext = nc.dram_tensor("Woext", [D, D + 2], BF16, kind="ExternalInput")
    outT = nc.dram_tensor("outT", [D, SLICE], F32, kind="ExternalOutput")

    with ExitStack() as ctx:
        tc = ctx.enter_context(tile.TileContext(nc))
        psum = ctx.enter_context(tc.tile_pool(name="psum", bufs=1, space="PSUM"))
        persist = ctx.enter_context(tc.tile_pool(name="persist", bufs=1))
        work = ctx.enter_context(tc.tile_pool(name="work", bufs=1))
        small = ctx.enter_context(tc.tile_pool(name="small", bufs=1))
        dram = ctx.enter_context(tc.tile_pool(name="dram", bufs=1, space="DRAM"))
        pools = {"psum": psum, "persist": persist, "work": work,
                 "small": small, "dram": dram}

        ones1 = persist.tile([1, P], F32, tag="ones1")
        nc.vector.memset(ones1[:], 1.0)
        ones16 = persist.tile([1, P], F16, tag="ones16")
        nc.vector.memset(ones16[:], 1.0)
        _CACHED.clear()
        _CACHED["ones1"] = ones1
        _CACHED["ones16"] = ones16
        bias_d = persist.tile([P, 1], F32, tag="bias_d")
        nc.vector.memset(bias_d[:], -C_DST)
        _CACHED["bias_d"] = bias_d
        ones64t = persist.tile([1, D], F32, tag="ones64t")
        nc.vector.memset(ones64t[:], 1.0)
        _CACHED["ones64t"] = ones64t

        for _rep in range(repeat):
            _emit_gat(nc, pools, xT, adjT, adjT2, Wext, Woext, outT)

    nc.compile()
    return nc


def _emit_gat(nc, pools, xT, adjT, adjT2, Wext, Woext, outT):
    psum, persist, work, small, dram = (pools["psum"], pools["persist"],
                                        pools["work"], pools["small"],
                                        pools["dram"])
    # ---- phase 1: x/W loads, f rows, R broadcast, per-tile weight prep ----
    wext_sb = []
    for kf in range(NKF):
        t = small.tile([P, D + 2], F16, tag=f"wext{kf}", name=f"wext_{kf}")
        nc.sync.dma_start(out=t[:], in_=Wext[kf * P:(kf + 1) * P, :])
        wext_sb.append(t)
    # xT in column-halves so the f rows / first node tiles are ready early
    xt_sb = []
    for kf in range(NKF):
        t = work.tile([P, N], F16, tag=f"xt{kf}", name=f"xt_{kf}")
        xt_sb.append(t)
    for half in range(2):
        for kf in range(NKF):
            nc.sync.dma_start(
                out=xt_sb[kf][:, half * 2048:(half + 1) * 2048],
                in_=xT[kf * P:(kf + 1) * P, half * 2048:(half + 1) * 2048])
    woext_sb = persist.tile([D, D + 2], BF16, tag="woext")
    nc.scalar.dma_start(out=woext_sb[:], in_=Woext[:])

    # f_src row (partition 0) via M=1 matmuls, staged per 512-chunk;
    # R_bc = exp(-(1-a) f_src) broadcast [P, N]. (f_dst reaches the per-tile
    # scalars via the staged wh_ps column instead of a row transpose.)
    R_bc = persist.tile([P, N], BF16, tag="R_l1", name="R_l1")

    def emit_Rbc(sl):
        fr_ps = psum.tile([1, 512], F32, tag="bank", bufs=8,
                          name=f"frps_{sl}")
        for kf in range(NKF):
            nc.tensor.matmul(fr_ps[:], wext_sb[kf][:, D:D + 1],
                             xt_sb[kf][:, sl * 512:(sl + 1) * 512],
                             start=(kf == 0), stop=(kf == NKF - 1))
        fs_ch = work.tile([1, 512], F16, tag="fsch", bufs=1,
                          name=f"fsch_{sl}")
        nc.scalar.activation(fs_ch[:], fr_ps[:],
                             mybir.ActivationFunctionType.Copy)
        bc_ps = psum.tile([P, 512], F32, tag="bank", bufs=8,
                          name=f"bc_l1_{sl}")
        nc.tensor.matmul(bc_ps[:], _CACHED["ones16"][0:1, :], fs_ch[:],
                         start=True, stop=True)
        nc.scalar.activation(R_bc[:, sl * 512:(sl + 1) * 512], bc_ps[:],
                             mybir.ActivationFunctionType.Exp,
                             scale=-(1.0 - ALPHA))

    whs1 = persist.tile([P, NT * (D + 1)], BF16, tag="whsbig_l1")
    nc.vector.memset(
        whs1[:].rearrange("p (t c) -> p t c", c=D + 1)[:, :, D:D + 1], 1.0)
    e1_1 = small.tile([P, NT], F32, tag="e1_l1")
    re1_1 = small.tile([P, NT], F32, tag="re1_l1")

    def emit_tile(t):
        wh_ps = psum.tile([P, D + 2], F32, tag="bank", bufs=8,
                          name=f"whps_{t}")
        for kf in range(NKF):
            nc.tensor.matmul(wh_ps[:], xt_sb[kf][:, t * P:(t + 1) * P],
                             wext_sb[kf][:], start=(kf == 0),
                             stop=(kf == NKF - 1))
        _tile_prep(nc, pools, "l1", t, wh_ps, whs1, e1_1, re1_1)

    # ALL R chunks first: their fs_ch ACT copies must not queue behind the
    # per-tile trios, or the in-order PE head-blocks on the later broadcast
    # matmuls and stalls the first piece's accumulation mid-stream
    for sl in range(8):
        emit_Rbc(sl)
    for t in range(NT):
        emit_tile(t)

    # ---- layer-1 attention pieces with overlapped exchange tails ----
    o1_sb = persist.tile([D, N], BF16, tag="o1")
    cc_in = [dram.tile([PIECES[h], D + 2], F16, tag=f"cc_in{h}",
                       name=f"cc_in{h}") for h in range(NP)]
    cc_rs = [dram.tile([RS_ROWS[h], D + 2], F16, tag=f"cc_rs{h}",
                       name=f"cc_rs{h}") for h in range(NP)]
    cc_full = [dram.tile([PIECES[h], D + 2], F16, tag=f"cc_full{h}",
                         addr_space="Shared", name=f"cc_full{h}")
               for h in range(NP)]

    def l1_adj_fn(h):
        lo = P_OFF[h]
        w = PIECES[h]

        def fn(j):
            adj_t = work.tile([P, w], BF16, tag=f"adjl1w{w}",
                              bufs=(5 if w <= 2048 else 3),
                              name=f"adj_l1_{h}_{j}")
            nc.sync.dma_start(out=adj_t[:],
                              in_=adjT[j * P:(j + 1) * P, lo:lo + w])
            return adj_t[:]
        return fn

    def l1_tail(h, on_pool):
        lo = P_OFF[h]
        t0 = P_OFF[h] // P
        # per 512-chunk: ELU then its 4 h@Wo tiles + stores, so at most a few
        # matmuls wait on PE at a time (wait-queue bypass keeps the next
        # piece's accumulation flowing)
        for s in range(PIECES[h] // 512):
            _elu(nc, pools, f"l1h{h}s{s}",
                 o1_sb[:, lo + s * 512:lo + (s + 1) * 512],
                 o1_sb[:, lo + s * 512:lo + (s + 1) * 512], 512, on_pool)
            for tt in range(4 * s, 4 * s + 4):
                t = t0 + tt
                p2_ps = psum.tile([P, D + 2], F32, tag="bank", bufs=8,
                                  name=f"p2ps_{t}")
                nc.tensor.matmul(p2_ps[:], o1_sb[:, t * P:(t + 1) * P],
                                 woext_sb[:], start=True, stop=True)
                p2_sb = work.tile([P, D + 2], F16, tag="stage66", bufs=4,
                                  name=f"p2sb_{t}")
                nc.scalar.activation(p2_sb[:], p2_ps[:],
                                     mybir.ActivationFunctionType.Copy)
                nc.scalar.dma_start(out=cc_in[h][tt * P:(tt + 1) * P, :],
                                    in_=p2_sb[:])
        nc.gpsimd.collective_compute(
            "ReduceScatter", mybir.AluOpType.add,
            ins=[cc_in[h][:]], outs=[cc_rs[h][:]],
            replica_groups=[list(range(N_CORES))])
        nc.gpsimd.collective_compute(
            "AllGather", mybir.AluOpType.bypass,
            ins=[cc_rs[h][:]], outs=[cc_full[h][:]],
            replica_groups=[list(range(N_CORES))])

    for h in range(NP):
        lo = P_OFF[h]
        accs = [psum.tile([D + 1, 512], F32, tag="bank", bufs=8,
                          name=f"acc_l1_{h}_{q}")
                for q in range(PIECES[h] // 512)]
        _att_rows(nc, pools, f"l1h{h}", whs1, e1_1, re1_1, R_bc, lo,
                  range(NT), l1_adj_fn(h), accs, 512, 0, NT - 1)
        if h == NP - 1:
            # prefetch ALL layer-2 adjacency while the DMA stream is free;
            # L2 attention then has no load dependency at all.
            for ch in range(NP):
                clo = RS_OFF[ch]
                for j in range(NT):
                    a2 = work.tile([P, RS_ROWS[ch]], BF16,
                                   tag=f"adjl2_{ch}_{j}", bufs=1,
                                   name=f"adj_l2_{ch}_{j}")
                    nc.sync.dma_start(
                        out=a2[:], in_=adjT2[j * P:(j + 1) * P,
                                             clo:clo + RS_ROWS[ch]])
                    _CACHED[f"adj2_{ch}_{j}"] = a2
        _norm(nc, pools, f"l1h{h}", accs, o1_sb, lo, 512,
              on_pool=(h < NP - 1))
        l1_tail(h, on_pool=(h < NP - 1))

    # ---- layer 2: work gated per exchange piece in both j and i ----
    whs2 = persist.tile([P, NT * (D + 1)], BF16, tag="whsbig_l2")
    nc.vector.memset(
        whs2[:].rearrange("p (t c) -> p t c", c=D + 1)[:, :, D:D + 1], 1.0)
    e1_2 = small.tile([P, NT], F32, tag="e1_l2")
    re1_2 = small.tile([P, NT], F32, tag="re1_l2")

    def l2_prep(h):
        t0 = sum(L2_JT[:h])
        for tt in range(L2_JT[h]):
            t = t0 + tt
            s = work.tile([P, D + 2], F16, tag="ccsb", bufs=4,
                          name=f"ccsb_{t}")
            nc.sync.dma_start(out=s[:],
                              in_=cc_full[h][tt * P:(tt + 1) * P, :])
            _tile_prep(nc, pools, "l2", t, s, whs2, e1_2, re1_2)

    def l2_Rbc(h):
        fs2 = small.tile([1, RS_ROWS[h]], F16, tag=f"fs2_{h}")
        nc.scalar.dma_start(
            out=fs2[:],
            in_=cc_rs[h][:, D:D + 1].rearrange("n one -> one n"))
        R2 = persist.tile([P, RS_ROWS[h]], BF16, tag=f"R2_{h}")
        bc_ps = psum.tile([P, RS_ROWS[h]], F32, tag="bank", bufs=8,
                          name=f"bc_l2_{h}")
        nc.tensor.matmul(bc_ps[:], _CACHED["ones16"][0:1, :], fs2[:],
                         start=True, stop=True)
        nc.scalar.activation(R2[:], bc_ps[:],
                             mybir.ActivationFunctionType.Exp,
                             scale=-(1.0 - ALPHA))
        return R2

    def l2_adj_fn(ch):
        lo = RS_OFF[ch]

        def fn(j):
            return _CACHED[f"adj2_{ch}_{j}"][:]
        return fn

    def jg(k):
        return range(sum(L2_JT[:k]), sum(L2_JT[:k + 1]))

    def l2_att(k, ch):
        _att_rows(nc, pools, f"l2_{k}_{ch}", whs2, e1_2, re1_2, R2[ch], 0,
                  jg(k), l2_adj_fn(ch), [acc2[ch]], RS_ROWS[ch], 0, NT - 1)

    acc2 = [psum.tile([D + 1, RS_ROWS[ch]], F32, tag="bank", bufs=8,
                      name=f"acc_l2_{ch}") for ch in range(NP)]
    R2 = [None] * NP
    o2_sb = persist.tile([D, SLICE], BF16, tag="o2")
    fin = persist.tile([D, SLICE], F32, tag="fin")

    # emission (= priority) ordered by dependency arrival: AG/RS of piece k
    # gate j-group k / column-slice k respectively
    l2_prep(0)
    R2[0] = l2_Rbc(0)
    l2_att(0, 0)
    if NP >= 2:
        R2[1] = l2_Rbc(1)
        l2_att(0, 1)
    for k in range(1, NP - 1):
        l2_prep(k)
        for ch in range(k + 1):
            l2_att(k, ch)
        R2[k + 1] = l2_Rbc(k + 1)
        for kk in range(k + 1):
            l2_att(kk, k + 1)
    l2_prep(NP - 1)
    for ch in range(NP):
        l2_att(NP - 1, ch)
        lo = RS_OFF[ch]
        _norm(nc, pools, f"l2_{ch}", [acc2[ch]], o2_sb, lo, RS_ROWS[ch],
              on_pool=False)
        _elu(nc, pools, f"l2_{ch}", o2_sb[:, lo:lo + RS_ROWS[ch]],
             fin[:, lo:lo + RS_ROWS[ch]], RS_ROWS[ch], on_pool=False,
             ew=RS_ROWS[ch])
        nc.sync.dma_start(out=outT[:, lo:lo + RS_ROWS[ch]],
                          in_=fin[:, lo:lo + RS_ROWS[ch]])


# ---------------------------------------------------------------------------
# host-side driver
# ---------------------------------------------------------------------------

def _prep_inputs(x, adj, W, a, Wo, ao):
    xT = np.ascontiguousarray(x.T.astype(np.float16))
    adjT = np.ascontiguousarray(adj.T.astype(ml_dtypes.bfloat16))
    in_maps = []
    for c in range(N_CORES):
        a_src, a_dst = a[c, :D], a[c, D:]
        wext = np.concatenate(
            [W[c], (W[c] @ a_src)[:, None], (W[c] @ a_dst)[:, None]],
            axis=1).astype(np.float16)
        Wo_h = Wo[c * D:(c + 1) * D]
        woext = np.concatenate(
            [Wo_h, (Wo_h @ ao[:D])[:, None], (Wo_h @ ao[D:])[:, None]],
            axis=1).astype(ml_dtypes.bfloat16)
        adjt2 = np.concatenate(
            [adjT[:, P_OFF[h] + c * RS_ROWS[h]:
                  P_OFF[h] + (c + 1) * RS_ROWS[h]]
             for h in range(NP)], axis=1)
        in_maps.append({
            "xT": xT,
            "adjT": adjT,
            "adjT2": np.ascontiguousarray(adjt2),
            "Wext": wext,
            "Woext": woext,
        })
    return in_maps


def kernel(x, adj, W, a, Wo, ao, cfg):
    x = np.asarray(x, np.float32)
    adj = np.asarray(adj, np.float32)
    W = np.asarray(W, np.float32)
    a = np.asarray(a, np.float32)
    Wo = np.asarray(Wo, np.float32)
    ao = np.asarray(ao, np.float32)

    in_maps = _prep_inputs(x, adj, W, a, Wo, ao)
    if _CACHED.get("nc") is None:
        nc = build_kernel()
        _CACHED["nc"] = nc
    res = run_bass_kernel_spmd(_CACHED["nc"], in_maps,
                               core_ids=list(range(N_CORES)))
    out = np.empty((N, D), np.float32)
    for c in range(N_CORES):
        oT = res.results[c]["outT"]
        for h in range(NP):
            out[P_OFF[h] + c * RS_ROWS[h]:
                P_OFF[h] + (c + 1) * RS_ROWS[h], :] = \
                oT[:, RS_OFF[h]:RS_OFF[h] + RS_ROWS[h]].T
    return out


if __name__ == "__main__":
    import reference as ref_mod
    inputs = {k: np.asarray(v) for k, v in ref_mod.setup_inputs().items()}
    expected = np.asarray(ref_mod.reference(**ref_mod.setup_inputs()))
    got = kernel(**inputs)
    err = np.abs(got - expected).max() / np.abs(expected).max()
    print("rel err:", err)
